# revision 17
# baseline (speedup 1.0000x reference)
"""Block-sparse attention on 8 Trainium2 NeuronCores (Bass/Tile SPMD kernel).

Sharding: batch*head_groups across the 8 cores. Core c handles batch c//4 and
heads [4*(c%4), 4*(c%4)+4). Projection weights are sliced per core host-side
(pre-transposed + bf16-cast); the [16,16] block mask specializes the compiled
program (only kept blocks are computed). Each core emits a partial output
(its 256-wide d-slice pushed through Wo) in bf16; the host sums the 4
partials per batch in fp32 and adds the bias.

v2 design notes (cost-model-driven):
  - matmul cost = out_free_cols * cycles_per_row; M and K are free. So the
    denominator rides along as extra stationary columns (ones) at no cost.
  - Normalization is fully engine-local (no DMA broadcasts): per (p,chunk)
    head A accumulates av in [65,512] PSUM (v|1 stationary, den at row 64);
    head B uses a [zeros|1@32|v] M=128 stationary so its dh rows land on
    partitions 64:128 (lane-matched to outTbf rows 64:128) with den at row
    32. DVE reciprocal on the den row (bf16), a K=1 ones-stationary matmul
    broadcasts the recip row across partitions, and one DVE multiply writes
    the normalized bf16 out^T tile.
  - Head-serial score fills ([128,1024] PSUM, 2-buf) keep PSUM inside 8
    banks: sc 2x2 + av 2 + aux(rb & late-proj pss) 2.
  - q/k projections for p=1 are emitted between attention chunks of p=0 so
    the scalar engine's exp stream starts ~40us earlier.
  - Output partials are written bf16 (halves output DMA).
"""

import time
from contextlib import ExitStack

import ml_dtypes
import numpy as np

import concourse.bass as bass
import concourse.tile as tile
from concourse import bacc, mybir
from concourse.ap import AP as APClass
from concourse.bass_utils import run_bass_kernel_spmd

BF16 = mybir.dt.bfloat16
F32 = mybir.dt.float32
bf16 = ml_dtypes.bfloat16

B, S, D, H = 2, 2048, 1024, 16
DH = 64
BLK = 128
NB = 16
NCORES = 8
HPC = H // (NCORES // B)   # 4 heads per core
E = HPC * DH               # 256 projection columns per core
KD = D // 128              # 8 contraction chunks
VW = 386                   # vv tile width: [vA|1|z|1|z|vB][vA2|1|z|1|z|vB2]
FILL = 1024                # score fill width (2 PSUM banks)
NCH = 4                    # query blocks per attention chunk
CHW = NCH * BLK            # 512 query columns per chunk

_nc_cache: dict = {}
last_run_info: dict = {}

I16 = mybir.dt.int16
# Schraudolph-style exp producing bf16 bit patterns: for x in [-20, 5],
# round(x * 2^7/ln2 + (127*2^7 - 7.4)) viewed as bf16 approximates e^x with
# ~2% rms relative error. Used on the DVE for half the score fills to take
# load off the (otherwise saturating) scalar engine's exact Exp.
SCH_A = 184.6649652337873       # 2^7 / ln 2
SCH_B = 16256.0 - 7.4           # 127 * 2^7 - c


def _strided_ap(sl, offsets_elems, inner_n):
    """AP over a [128, W] SBUF slice selecting `inner_n` consecutive elements
    at each offset in `offsets_elems` (uniform stride required)."""
    apl = [list(x) for x in sl.ap]
    assert len(apl) == 2, apl
    elem = apl[1][0]
    stride = offsets_elems[1] - offsets_elems[0] if len(offsets_elems) > 1 else 1
    for a, b in zip(offsets_elems, offsets_elems[1:]):
        assert b - a == stride
    return APClass(
        sl.tensor,
        sl.offset + offsets_elems[0] * elem,
        [apl[0], [stride * elem, len(offsets_elems)], [elem, inner_n]],
    )


def _runs_of(lst):
    out = []
    for i in lst:
        if out and i == out[-1][-1] + 1:
            out[-1].append(i)
        else:
            out.append([i])
    return out


def _emit(tc, aps, kept):
    nc = tc.nc
    xT_ap, wqT_ap, wkT_ap, wvT_ap, woT_ap, outp_ap = aps
    Exp = mybir.ActivationFunctionType.Exp

    col_kept = [[i for i in range(NB) if j in kept[i]] for j in range(NB)]
    last_j = {i: kept[i][-1] for i in range(NB)}

    with ExitStack() as ctx:
        persist = ctx.enter_context(tc.tile_pool(name="persist", bufs=1))

        # ---- input loads ------------------------------------------------------
        xT = []
        for kd in range(KD):
            t = persist.tile([128, S], BF16, name=f"xT{kd}", tag=f"xT{kd}")
            xT.append(t)
        wq = persist.tile([128, KD * E], BF16, name="wq", tag="wq")
        wk = persist.tile([128, KD * E], BF16, name="wk", tag="wk")
        wv = persist.tile([128, KD * E], BF16, name="wv", tag="wv")

        def load_w(t, src_ap):
            nc.sync.dma_start(
                t[:].rearrange("p (k e) -> p k e", k=KD),
                src_ap.rearrange("(k p) e -> p k e", p=128),
            )

        # DMA order tuned so q-proj (wq + xT chunks) can start ASAP and k-proj
        # shortly after. First pieces are small to cut the startup latency.
        nc.sync.dma_start(wq[:, 0:E], wqT_ap[0:128, :])
        nc.sync.dma_start(xT[0][:, 0:512], xT_ap[0:128, 0:512])
        nc.sync.dma_start(
            wq[:, E:].rearrange("p (k e) -> p k e", k=KD - 1),
            wqT_ap[128:, :].rearrange("(k p) e -> p k e", p=128),
        )
        nc.sync.dma_start(xT[0][:, 512:], xT_ap[0:128, 512:])
        nc.sync.dma_start(xT[1][:], xT_ap[128:256, :])
        load_w(wk, wkT_ap)
        for kd in range(2, KD):
            nc.sync.dma_start(xT[kd][:], xT_ap[kd * 128:(kd + 1) * 128, :])
        load_w(wv, wvT_ap)
        wo = []
        for p in range(2):
            t = persist.tile([128, D], BF16, name=f"wo{p}", tag=f"wo{p}")
            nc.sync.dma_start(t[:], woT_ap[p * 128:(p + 1) * 128, :])
            wo.append(t)

        def wsl(w, kd, p):
            return w[:, kd * E + p * 128: kd * E + (p + 1) * 128]

        zeros_sb = persist.tile([128, 512], BF16, name="zeros_sb", tag="zeros_sb")
        nc.gpsimd.memset(zeros_sb[:], 0.0)
        ones_bf = persist.tile([128, 128], BF16, name="ones_bf", tag="ones_bf")
        nc.gpsimd.memset(ones_bf[:], 1.0)

        qT = [persist.tile([128, S], BF16, name=f"qT{p}", tag=f"qT{p}") for p in range(2)]
        kT = [persist.tile([128, S], BF16, name=f"kT{p}", tag=f"kT{p}") for p in range(2)]
        # vv layout per m (386 cols):
        #   [0:64 v_h0][64 one][65:97 z][97 one][98:129 z][129:193 v_h1]
        #   [193:257 v_h2][257 one][258:290 z][290 one][291:322 z][322:386 v_h3]
        # A-slot(p) = 193p..193p+65 (v|1, M=65, out rows 0:65)
        # B-slot(p) = 65+193p..193+193p (z|1@32|z|v, M=128, out rows 64:128, den 32)
        vv = [persist.tile([128, VW], BF16, name=f"v{m}", tag=f"v{m}") for m in range(S // 128)]
        outTbf = [persist.tile([128, S], BF16, name=f"oT{p}", tag=f"oT{p}") for p in range(2)]

        # vv constant regions (zeros under B slots, ones for den columns)
        for m in range(S // 128):
            nc.gpsimd.memset(_strided_ap(vv[m][:], [65, 258], 64), 0.0)
            nc.gpsimd.memset(_strided_ap(vv[m][:], [64, 97], 1), 1.0)
            nc.gpsimd.memset(_strided_ap(vv[m][:], [257, 290], 1), 1.0)

        # ---- projections for p=0 and v ---------------------------------------
        with ExitStack() as pctx:
            proj_ps = pctx.enter_context(tc.tile_pool(name="proj_ps", bufs=4, space="PSUM"))
            vproj_ps = pctx.enter_context(tc.tile_pool(name="vproj_ps", bufs=4, space="PSUM"))

            def qk_proj(dst, w, p, copy_fn):
                pss = [proj_ps.tile([128, 512], F32, name="projps", tag="proj") for _ in range(4)]
                for kd in range(KD):
                    for sc in range(4):
                        nc.tensor.matmul(
                            pss[sc][:],
                            wsl(w, kd, p),
                            xT[kd][:, sc * 512:(sc + 1) * 512],
                            start=(kd == 0),
                            stop=(kd == KD - 1),
                        )
                for sc in range(4):
                    copy_fn(dst[p][:, sc * 512:(sc + 1) * 512], pss[sc][:])

            def v_proj(m):
                ps = vproj_ps.tile([128, 256], F32, name="projv", tag="vproj")
                for kd in range(KD):
                    nc.tensor.matmul(
                        ps[:],
                        xT[kd][:, m * 128:(m + 1) * 128],
                        wv[:, kd * E:(kd + 1) * E],
                        start=(kd == 0),
                        stop=(kd == KD - 1),
                    )
                # A-heads (h0 -> 0:64, h2 -> 193:257), B-heads (h1 -> 129:193, h3 -> 322:386)
                nc.vector.tensor_copy(
                    _strided_ap(vv[m][:], [0, 193], 64),
                    _strided_ap(ps[:], [0, 128], 64),
                )
                nc.scalar.copy(
                    _strided_ap(vv[m][:], [129, 322], 64),
                    _strided_ap(ps[:], [64, 192], 64),
                )

            qk_proj(qT, wq, 0, nc.vector.tensor_copy)
            qk_proj(kT, wk, 0, nc.scalar.copy)
            for m in range(S // 128):
                v_proj(m)

        # ---- attention + late projections ------------------------------------
        actx = ExitStack()
        sc_pool = actx.enter_context(tc.tile_pool(name="sc_ps", bufs=2, space="PSUM"))
        av_pool = actx.enter_context(tc.tile_pool(name="av_ps", bufs=2, space="PSUM"))
        aux_pool = actx.enter_context(tc.tile_pool(name="aux_ps", bufs=2, space="PSUM"))
        at_pool = actx.enter_context(tc.tile_pool(name="at_sb", bufs=3))
        rec_pool = actx.enter_context(tc.tile_pool(name="rec_sb", bufs=2))

        def late_qk_proj(dst, w, p, copy_fn):
            for sc in range(4):
                ps = aux_pool.tile([128, 512], F32, name="lateps", tag="aux")
                for kd in range(KD):
                    nc.tensor.matmul(
                        ps[:],
                        wsl(w, kd, p),
                        xT[kd][:, sc * 512:(sc + 1) * 512],
                        start=(kd == 0),
                        stop=(kd == KD - 1),
                    )
                copy_fn(dst[p][:, sc * 512:(sc + 1) * 512], ps[:])

        def attention_chunk(p, ch):
            irange = range(ch * NCH, ch * NCH + NCH)
            avA = av_pool.tile([65, CHW], F32, name="avA", tag="av")
            avB = av_pool.tile([128, CHW], F32, name="avB", tag="av")
            # zero-prime both av banks so region accumulation uses start=False
            nc.tensor.matmul(avA[:], vv[0][:, 193 * p:193 * p + 65], zeros_sb[:],
                             start=True, stop=False, skip_group_check=True)
            nc.tensor.matmul(avB[:], vv[0][:, 65 + 193 * p:193 + 193 * p], zeros_sb[:],
                             start=True, stop=False, skip_group_check=True)

            nfill = [0]
            for a in range(2):  # head within pair, serial
                rows = slice(64 * a, 64 * a + 64)

                def flush(fill_js, sc, colw):
                    at = at_pool.tile([128, FILL], BF16, name="at", tag="at")
                    if nfill[0] % 2 == 0:
                        nc.scalar.activation(at[:, 0:colw], sc[:, 0:colw], Exp)
                    else:
                        nc.vector.tensor_scalar(
                            at[:, 0:colw].bitcast(I16), sc[:, 0:colw],
                            SCH_A, SCH_B, mybir.AluOpType.mult, mybir.AluOpType.add,
                        )
                    nfill[0] += 1
                    for j, ks, off in fill_js:
                        lhs = (vv[j][:, 193 * p:193 * p + 65] if a == 0
                               else vv[j][:, 65 + 193 * p:193 + 193 * p])
                        for run in _runs_of(ks):
                            # split where the stop flag changes
                            sub = []
                            for i in run:
                                fl = (j == last_j[i])
                                if sub and sub[-1][1] == fl:
                                    sub[-1][0].append(i)
                                else:
                                    sub.append(([i], fl))
                            for seg, fl in sub:
                                il0 = seg[0] - ch * NCH
                                c0 = off + ks.index(seg[0]) * 128
                                w = len(seg) * 128
                                out = (avA[0:65, il0 * 128: il0 * 128 + w] if a == 0
                                       else avB[:, il0 * 128: il0 * 128 + w])
                                nc.tensor.matmul(
                                    out, lhs, at[:, c0:c0 + w],
                                    start=False, stop=fl, skip_group_check=True,
                                )

                fill_js = []
                sc = None
                colw = 0
                for j in range(NB):
                    ks = [i for i in col_kept[j] if i in irange]
                    if not ks:
                        continue
                    n = len(ks)
                    if sc is None or colw + n * 128 > FILL:
                        if sc is not None:
                            flush(fill_js, sc, colw)
                        sc = sc_pool.tile([128, FILL], F32, name="sc", tag="sc")
                        fill_js = []
                        colw = 0
                    for run in _runs_of(ks):
                        idx0 = ks.index(run[0])
                        col = colw + idx0 * 128
                        width = len(run) * 128
                        qcol = run[0] * 128
                        done = 0
                        while done < width:
                            seg = min(width - done, 512 - ((col + done) % 512))
                            nc.tensor.matmul(
                                sc[:, col + done: col + done + seg],
                                kT[p][rows, j * 128:(j + 1) * 128],
                                qT[p][rows, qcol + done: qcol + done + seg],
                            )
                            done += seg
                    fill_js.append((j, ks, colw))
                    colw += n * 128
                if sc is not None:
                    flush(fill_js, sc, colw)

            # ---- normalization (engine-local, no DMA) ----
            rec = rec_pool.tile([128, CHW], BF16, name="rec", tag="rec")
            with nc.allow_low_precision("softmax denom recip in bf16"):
                nc.vector.reciprocal(rec[64:65, :], avA[64:65, :])
                nc.vector.reciprocal(rec[32:33, :], avB[32:33, :])
            rb = aux_pool.tile([128, CHW], F32, name="rb", tag="aux")
            nc.tensor.matmul(rb[0:64, :], ones_bf[64:65, 0:64], rec[64:65, :],
                             skip_group_check=True)
            nc.tensor.matmul(rb[64:128, :], ones_bf[32:33, 0:64], rec[32:33, :],
                             skip_group_check=True)
            # engines may read only one PSUM operand per op: stage rb in SBUF
            rbs = rec_pool.tile([128, CHW], BF16, name="rbs", tag="rbs")
            nc.vector.tensor_copy(rbs[:], rb[:])
            ocols = slice(ch * CHW, (ch + 1) * CHW)
            nc.vector.tensor_mul(outTbf[p][0:64, ocols], avA[0:64, :], rbs[0:64, :])
            nc.vector.tensor_mul(outTbf[p][64:128, ocols], avB[64:128, :], rbs[64:128, :])

        fin_sb = actx.enter_context(tc.tile_pool(name="fin_sb", bufs=4))

        def final_group(ch):
            # output projection for chunk ch's 4 query blocks; PSUM from
            # aux_pool ([128,512] tiles, two per m)
            for m in range(ch * NCH, (ch + 1) * NCH):
                st = fin_sb.tile([128, D], BF16, name="finst", tag="finsb")
                for n in range(2):
                    ps = aux_pool.tile([128, 512], F32, name="finps", tag="aux")
                    for p in range(2):
                        nc.tensor.matmul(
                            ps[:],
                            outTbf[p][:, m * 128:(m + 1) * 128],
                            wo[p][:, n * 512:(n + 1) * 512],
                            start=(p == 0),
                            stop=(p == 1),
                        )
                    if n == 0:
                        nc.vector.tensor_copy(st[:, 0:512], ps[:])
                    else:
                        nc.scalar.copy(st[:, 512:1024], ps[:])
                nc.sync.dma_start(outp_ap[m * 128:(m + 1) * 128, :], st[:])

        attention_chunk(0, 0)
        late_qk_proj(qT, wq, 1, nc.vector.tensor_copy)
        attention_chunk(0, 1)
        late_qk_proj(kT, wk, 1, nc.vector.tensor_copy)
        attention_chunk(0, 2)
        attention_chunk(0, 3)
        attention_chunk(1, 0)
        attention_chunk(1, 1)
        final_group(0)
        attention_chunk(1, 2)
        final_group(1)
        attention_chunk(1, 3)
        final_group(2)
        final_group(3)
        actx.close()


def _get_nc(kept):
    key = kept
    if key in _nc_cache:
        return _nc_cache[key]
    nc = bacc.Bacc("TRN2", target_bir_lowering=False, debug=False, num_devices=NCORES)
    xT_ap = nc.dram_tensor("xT", [D, S], BF16, kind="ExternalInput").ap()
    wqT_ap = nc.dram_tensor("wqT", [D, E], BF16, kind="ExternalInput").ap()
    wkT_ap = nc.dram_tensor("wkT", [D, E], BF16, kind="ExternalInput").ap()
    wvT_ap = nc.dram_tensor("wvT", [D, E], BF16, kind="ExternalInput").ap()
    woT_ap = nc.dram_tensor("woT", [E, D], BF16, kind="ExternalInput").ap()
    outp_ap = nc.dram_tensor("outp", [S, D], BF16, kind="ExternalOutput").ap()
    with tile.TileContext(nc) as tc:
        _emit(tc, (xT_ap, wqT_ap, wkT_ap, wvT_ap, woT_ap, outp_ap), kept)
    nc.compile()
    _nc_cache[key] = nc
    return nc


def kernel(x, Wq, Wk, Wv, Wo, bo, block_mask):
    x = np.asarray(x, dtype=np.float32)
    Wq = np.asarray(Wq, dtype=np.float32)
    Wk = np.asarray(Wk, dtype=np.float32)
    Wv = np.asarray(Wv, dtype=np.float32)
    Wo = np.asarray(Wo, dtype=np.float32)
    bo = np.asarray(bo, dtype=np.float32)
    mask = np.asarray(block_mask).astype(bool)

    kept = tuple(tuple(int(j) for j in np.nonzero(mask[i])[0]) for i in range(NB))
    assert all(len(js) > 0 for js in kept), "a query block row has no kept blocks"

    t0 = time.monotonic()
    nc = _get_nc(kept)
    t_compile = time.monotonic() - t0

    xT_b = [np.ascontiguousarray(x[b].T).astype(bf16) for b in range(B)]
    in_maps = []
    for c in range(NCORES):
        b = c // (NCORES // B)
        hs = c % (NCORES // B)
        sl = slice(hs * E, (hs + 1) * E)
        in_maps.append({
            "xT": xT_b[b],
            "wqT": np.ascontiguousarray((Wq[sl, :] / np.sqrt(np.float32(DH))).T).astype(bf16),
            "wkT": np.ascontiguousarray(Wk[sl, :].T).astype(bf16),
            "wvT": np.ascontiguousarray(Wv[sl, :].T).astype(bf16),
            "woT": np.ascontiguousarray(Wo[:, sl].T).astype(bf16),
        })

    t0 = time.monotonic()
    res = run_bass_kernel_spmd(nc, in_maps, list(range(NCORES)))
    t_run = time.monotonic() - t0

    out = np.zeros((B, S, D), np.float32)
    for c in range(NCORES):
        out[c // (NCORES // B)] += res.results[c]["outp"].astype(np.float32)
    out += bo[None, None, :]

    last_run_info.update(compile_s=t_compile, run_s=t_run, nc=nc)
    return out
